# revision 5
# baseline (speedup 1.0000x reference)
"""MoE layer (router + 8 experts top-2 + shared expert) on 8 Trainium2 cores.

Strategy (expert-parallel, matching the all-to-all dispatch hint):
  - Host computes router logits/top-2/softmax and gathers each expert's
    tokens (the "all-to-all dispatch" — host-side since kernel() owns the
    full inputs and sharding).
  - Core c holds expert c's W1/W2 and computes
        y_c = relu(x_gathered @ W1_c + b1_c) @ W2_c
    for its (padded-to-capacity) token set, in transposed layout so both
    matmul stationary operands come straight from the natural weight layout.
  - The always-on shared expert is d_ff-sliced 8 ways: core c computes
    partial_c = relu(x_all @ Ws1[:, c*512:(c+1)*512] + bs1[slice]) @ Ws2[slice]
    over all tokens; partials are summed on host.
  - Host applies gate weights, b2/bs2 biases, and scatter-adds expert
    outputs back to token order.

All matmuls run as float32r (full-rate fp32 path on the PE array) with
fp32 PSUM accumulation.
"""

import os
import sys

import numpy as np

for _p in ("/opt/trn_rl_repo", os.path.expanduser("~/.axon_site/_ro/trn_rl_repo")):
    if os.path.isdir(_p) and _p not in sys.path:
        sys.path.append(_p)

import concourse.bass as bass  # noqa: E402
import concourse.tile as tile  # noqa: E402
from concourse import bacc, mybir  # noqa: E402
from concourse.bass import ds, ts  # noqa: E402
from concourse.bass_utils import run_bass_kernel_spmd  # noqa: E402

D_MODEL, D_FF, N_EXP, TOP_K = 1024, 4096, 8, 2
P = 128
KD = D_MODEL // P        # 8 partition-tiles over d_model
MF = D_FF // P           # 32 partition-tiles over d_ff
FF_SH = D_FF // N_EXP    # 512: shared-expert d_ff slice per core
MS = FF_SH // P          # 4 partition-tiles over the shared slice
T_TOTAL = 4096
TT = 1024                # shared-expert token tile

F32 = mybir.dt.float32
F32R = mybir.dt.float32r
RELU = mybir.ActivationFunctionType.Relu
ADD = mybir.AluOpType.add


def _col_slices(n, step=512):
    """Near-equal chunks of at most `step` columns.  Keeping every chunk
    >= 256 matters: float32r matmuls with a moving dim below 256 run at
    1/4 rate on the PE array, so a ragged 512/512/68 split wastes ~50us
    per iteration vs 364/364/364."""
    if n <= step:
        return [(0, n)]
    cnt = -(-n // step)
    w = -(-n // (cnt * 4)) * 4
    out, off = [], 0
    while off + w < n:
        out.append((off, w))
        off += w
    out.append((off, n - off))
    return out


def _declare_io(nc, C, external_inputs=True):
    if external_inputs:
        def inp(name, shape, dt):
            return nc.declare_dram_parameter(name, shape, dt, isOutput=False)
    else:
        def inp(name, shape, dt):
            return nc.dram_tensor(name, shape, dt)
    t = {}
    t["xg"] = inp("xg", [P, KD, C], F32R)
    t["xt"] = inp("xt", [P, KD, T_TOTAL], F32R)
    t["w1"] = inp("w1", [P, KD, D_FF], F32R)
    t["w2"] = inp("w2", [P, MF, D_MODEL], F32R)
    t["b1t"] = inp("b1t", [P, MF], F32)
    t["ws1"] = inp("ws1", [P, KD, FF_SH], F32R)
    t["ws2"] = inp("ws2", [P, MS, D_MODEL], F32R)
    t["bs1t"] = inp("bs1t", [P, MS], F32)
    t["yt"] = nc.declare_dram_parameter("yt", [P, KD, C], F32, isOutput=True)
    t["st"] = nc.declare_dram_parameter("st", [P, KD, T_TOTAL], F32, isOutput=True)
    return t


NO_DMA = False


def _emit_body(nc, tc, t, C):
    if NO_DMA:
        class _Skip:
            def dma_start(self, *a, **k):
                return None
        sync_engine = _Skip()
    else:
        sync_engine = nc.sync
    xg, xt, w1, w2, b1t, ws1, ws2, bs1t, yt, st = (
        t["xg"], t["xt"], t["w1"], t["w2"], t["b1t"],
        t["ws1"], t["ws2"], t["bs1t"], t["yt"], t["st"],
    )
    ncs = _col_slices(C)
    with tc.tile_pool(name="ws_keep", bufs=1) as ws_keep:
        ws1_sb = ws_keep.tile([P, KD, FF_SH], F32R, tag="ws1")
        ws2_sb = ws_keep.tile([P, MS, D_MODEL], F32R, tag="ws2")
        bs1_sb = ws_keep.tile([P, MS], F32, tag="bs1")
        if True:
            # ---------------- Phase 1: this core's expert on gathered tokens
            with (
                tc.tile_pool(name="const1", bufs=1) as const1,
                tc.tile_pool(name="xg_p", bufs=1) as xg_p,
                tc.tile_pool(name="y_p", bufs=1) as y_p,
                tc.tile_pool(name="w1_p", bufs=2) as w1_p,
                tc.tile_pool(name="w2_p", bufs=2) as w2_p,
                tc.tile_pool(name="h_p", bufs=2) as h_p,
                tc.tile_pool(name="ph", bufs=5, space="PSUM") as ph,
                tc.tile_pool(name="py", bufs=3, space="PSUM") as py,
            ):
                b1_sb = const1.tile([P, MF], F32)
                sync_engine.dma_start(out=b1_sb[:], in_=b1t[:])
                xg_sb = xg_p.tile([P, KD, C], F32R)
                y_sb = y_p.tile([P, KD, C], F32)

                first_w1 = w1_p.tile([P, KD, 512], F32R, tag="w1", name="first_w1")
                for k in range(KD):
                    sync_engine.dma_start(out=first_w1[:, k, :], in_=w1[:, k, ds(0, 512)])
                    for off, ln in ncs:
                        sync_engine.dma_start(
                            out=xg_sb[:, k, ds(off, ln)], in_=xg[:, k, ds(off, ln)]
                        )

                for mg in range(MF // 4):  # 8 groups of 4 ff-tiles (512 ff)
                    if mg == 0:
                        w1_sb = first_w1
                    else:
                        w1_sb = w1_p.tile([P, KD, 512], F32R, tag="w1")
                        for k in range(KD):
                            sync_engine.dma_start(
                                out=w1_sb[:, k, :], in_=w1[:, k, ds(mg * 512, 512)]
                            )
                    w2_sb = w2_p.tile([P, 4, D_MODEL], F32R, tag="w2")
                    for q in range(4):
                        sync_engine.dma_start(
                            out=w2_sb[:, q, :], in_=w2[:, mg * 4 + q, :]
                        )
                    if mg == MF // 4 - 2:  # prefetch shared-expert weights
                        for k in range(KD):
                            sync_engine.dma_start(out=ws1_sb[:, k, :], in_=ws1[:, k, :])
                        for q in range(MS):
                            sync_engine.dma_start(out=ws2_sb[:, q, :], in_=ws2[:, q, :])
                        sync_engine.dma_start(out=bs1_sb[:], in_=bs1t[:])
                    h_sb = h_p.tile([P, 4, C], F32R, tag="h")

                    for m4 in range(4):
                        m = mg * 4 + m4
                        psums = [ph.tile([P, ln], F32, tag="ph", name=f"ph_{i}") for i, (_, ln) in enumerate(ncs)]
                        for k in range(KD):
                            lhsT = w1_sb[:, k, ts(m4, P)]
                            for i, (off, ln) in enumerate(ncs):
                                nc.tensor.matmul(
                                    psums[i][:],
                                    lhsT,
                                    xg_sb[:, k, ds(off, ln)],
                                    start=(k == 0),
                                    stop=(k == KD - 1),
                                )
                        for i, (off, ln) in enumerate(ncs):
                            nc.scalar.activation(
                                out=h_sb[:, m4, ds(off, ln)],
                                in_=psums[i][:],
                                func=RELU,
                                bias=b1_sb[:, m : m + 1],
                            )

                    for j in range(KD):
                        ypsums = [py.tile([P, ln], F32, tag="py", name=f"py_{i}") for i, (_, ln) in enumerate(ncs)]
                        for m4 in range(4):
                            lhsT = w2_sb[:, m4, ts(j, P)]
                            for i, (off, ln) in enumerate(ncs):
                                nc.tensor.matmul(
                                    ypsums[i][:],
                                    lhsT,
                                    h_sb[:, m4, ds(off, ln)],
                                    start=(m4 == 0),
                                    stop=(m4 == 3),
                                )
                        for i, (off, ln) in enumerate(ncs):
                            if mg == 0:
                                nc.scalar.copy(
                                    out=y_sb[:, j, ds(off, ln)], in_=ypsums[i][:]
                                )
                            else:
                                nc.vector.tensor_tensor(
                                    out=y_sb[:, j, ds(off, ln)],
                                    in0=y_sb[:, j, ds(off, ln)],
                                    in1=ypsums[i][:],
                                    op=ADD,
                                )
                        if mg == MF // 4 - 1:
                            sync_engine.dma_start(out=yt[:, j, :], in_=y_sb[:, j, :])

            # ---------------- Phase 2: shared expert, d_ff slice, all tokens
            with (
                tc.tile_pool(name="xt_p", bufs=2) as xt_p,
                tc.tile_pool(name="hs_p", bufs=2) as hs_p,
                tc.tile_pool(name="so_p", bufs=2) as so_p,
                tc.tile_pool(name="ph2", bufs=5, space="PSUM") as ph2,
                tc.tile_pool(name="py2", bufs=3, space="PSUM") as py2,
            ):
                tt_widths = [1024, 1024, 1024, 512, 512]
                tt_off = 0
                for ttw in tt_widths:
                    base = tt_off
                    tt_off += ttw
                    xt_sb = xt_p.tile([P, KD, TT], F32R, tag="xt")
                    for k in range(KD):
                        sync_engine.dma_start(
                            out=xt_sb[:, k, ds(0, ttw)], in_=xt[:, k, ds(base, ttw)]
                        )
                    hs_sb = hs_p.tile([P, MS, TT], F32R, tag="hs")
                    for m in range(MS):
                        psums = [ph2.tile([P, 512], F32, tag="ph2", name=f"ph2_{n}") for n in range(ttw // 512)]
                        for k in range(KD):
                            lhsT = ws1_sb[:, k, ts(m, P)]
                            for n in range(ttw // 512):
                                nc.tensor.matmul(
                                    psums[n][:],
                                    lhsT,
                                    xt_sb[:, k, ds(n * 512, 512)],
                                    start=(k == 0),
                                    stop=(k == KD - 1),
                                )
                        for n in range(ttw // 512):
                            nc.scalar.activation(
                                out=hs_sb[:, m, ds(n * 512, 512)],
                                in_=psums[n][:],
                                func=RELU,
                                bias=bs1_sb[:, m : m + 1],
                            )
                    s_sb = so_p.tile([P, KD, TT], F32, tag="so")
                    for j in range(KD):
                        ypsums = [py2.tile([P, 512], F32, tag="py2", name=f"py2_{n}") for n in range(ttw // 512)]
                        for m in range(MS):
                            lhsT = ws2_sb[:, m, ts(j, P)]
                            for n in range(ttw // 512):
                                nc.tensor.matmul(
                                    ypsums[n][:],
                                    lhsT,
                                    hs_sb[:, m, ds(n * 512, 512)],
                                    start=(m == 0),
                                    stop=(m == MS - 1),
                                )
                        for n in range(ttw // 512):
                            nc.scalar.copy(
                                out=s_sb[:, j, ds(n * 512, 512)], in_=ypsums[n][:]
                            )
                        sync_engine.dma_start(
                            out=st[:, j, ds(base, ttw)], in_=s_sb[:, j, ds(0, ttw)]
                        )



def build_program(C):
    nc = bacc.Bacc(None, target_bir_lowering=False, debug=False)
    t = _declare_io(nc, C, external_inputs=True)
    with tile.TileContext(nc) as tc:
        _emit_body(nc, tc, t, C)
    nc.compile()
    return nc


def build_timing_program(C, trip):
    """Timing variant: inputs are Internal DRAM (no host transfer), body
    repeated `trip` times in a hardware loop."""
    nc = bacc.Bacc(None, target_bir_lowering=False, debug=False)
    t = _declare_io(nc, C, external_inputs=False)
    with tile.TileContext(nc) as tc:
        with tc.For_i(0, trip, 1):
            _emit_body(nc, tc, t, C)
    nc.compile()
    return nc


def _to_tiles(a2d):
    """[R, N] with R = r_tiles*128 -> [128, r_tiles, N] so element
    [p, r, n] = a2d[r*128 + p, n]; contiguous for a single straight DMA."""
    R, N = a2d.shape
    return np.ascontiguousarray(
        a2d.reshape(R // P, P, N).transpose(1, 0, 2)
    )


def _from_tiles(a3d):
    """Inverse of _to_tiles: [128, r_tiles, N] -> [r_tiles*128, N]."""
    p, r, n = a3d.shape
    return a3d.transpose(1, 0, 2).reshape(r * p, n)


def _route(xf, Wg):
    """Replicates TopKRouter eval: top-2 by logit, softmax over the two."""
    logits = xf @ Wg
    top_idx = np.argsort(-logits, axis=1, kind="stable")[:, :TOP_K]
    top_vals = np.take_along_axis(logits, top_idx, axis=1)
    e = np.exp(top_vals - top_vals.max(axis=1, keepdims=True))
    top_w = (e / e.sum(axis=1, keepdims=True)).astype(np.float32)
    return top_idx, top_w


_PROG_CACHE = {}


def _get_program(C):
    if C not in _PROG_CACHE:
        _PROG_CACHE[C] = build_program(C)
    return _PROG_CACHE[C]


def make_in_maps(x, Wg, W1, b1, W2, b2, Ws1, bs1, Ws2, bs2):
    """Host-side routing + sharding. Returns (in_maps, C, idx_e, gate_e, xf)."""
    B, S, D = x.shape
    T = B * S
    xf = np.ascontiguousarray(np.asarray(x, np.float32).reshape(T, D))
    top_idx, top_w = _route(xf, np.asarray(Wg, np.float32))

    idx_e, gate_e = [], []
    for ex in range(N_EXP):
        rows, slot = np.nonzero(top_idx == ex)
        idx_e.append(rows)
        gate_e.append(top_w[rows, slot])
    counts = [len(i) for i in idx_e]
    C = max(4, -(-max(counts) // 4) * 4)

    xt_tiled = _to_tiles(xf.T)  # [128, 8, 4096]
    in_maps = []
    for ex in range(N_EXP):
        xg = np.zeros((C, D_MODEL), np.float32)
        xg[: counts[ex]] = xf[idx_e[ex]]
        sl = slice(ex * FF_SH, (ex + 1) * FF_SH)
        in_maps.append(
            {
                "xg": _to_tiles(np.ascontiguousarray(xg.T)),
                "xt": xt_tiled,
                "w1": _to_tiles(np.asarray(W1[ex], np.float32)),
                "w2": _to_tiles(np.asarray(W2[ex], np.float32)),
                "b1t": np.ascontiguousarray(
                    np.asarray(b1[ex], np.float32).reshape(MF, P).T
                ),
                "ws1": _to_tiles(np.asarray(Ws1[:, sl], np.float32)),
                "ws2": _to_tiles(np.asarray(Ws2[sl, :], np.float32)),
                "bs1t": np.ascontiguousarray(
                    np.asarray(bs1[sl], np.float32).reshape(MS, P).T
                ),
            }
        )
    return in_maps, C, idx_e, gate_e, xf


def assemble_output(results, shape, C, idx_e, gate_e, b2, bs2):
    B, S, D = shape
    T = B * S
    out = np.zeros((T, D), np.float32)
    for ex in range(N_EXP):
        out += _from_tiles(results[ex]["st"]).T  # shared partials
    out += np.asarray(bs2, np.float32)[None, :]
    b2 = np.asarray(b2, np.float32)
    for ex in range(N_EXP):
        y = _from_tiles(results[ex]["yt"]).T[: len(idx_e[ex])]
        out[idx_e[ex]] += gate_e[ex][:, None] * (y + b2[ex][None, :])
    return out.reshape(B, S, D)


def kernel(x, Wg, W1, b1, W2, b2, Ws1, bs1, Ws2, bs2):
    in_maps, C, idx_e, gate_e, _ = make_in_maps(
        x, Wg, W1, b1, W2, b2, Ws1, bs1, Ws2, bs2
    )
    nc = _get_program(C)
    res = run_bass_kernel_spmd(nc, in_maps, list(range(N_EXP)))
    return assemble_output(
        res.results, x.shape, C, idx_e, gate_e, b2, bs2
    ).astype(np.float32)

